# revision 1
# baseline (speedup 1.0000x reference)
import sys

sys.path.insert(0, '/opt/trn_rl_repo')

import numpy as np
import concourse.bass as bass
from concourse import bacc
import concourse.mybir as mybir
import concourse.tile as tile
from concourse.masks import make_identity

S = 4096
H = 1024
I_ = 4096
NH = 16
HD = 64
NC = 8
SM = S // NC
DM = 128
HC = H // 128
ST = S // 512
F32 = mybir.dt.float32
F32R = mybir.dt.float32r
BF16 = mybir.dt.bfloat16
AF = mybir.ActivationFunctionType
ALU = mybir.AluOpType
AXX = mybir.AxisListType.X

_CACHE = {}


def build_nc():
    nc = bacc.Bacc(None, target_bir_lowering=False, debug=False)
    P = lambda name, shape: nc.declare_dram_parameter(name, shape, F32, isOutput=False)
    x_m = P("x_m", [SM, H])
    wqkvT = P("wqkvT", [H, 3 * DM])
    bqkv = P("bqkv", [1, 3 * DM])
    owT = P("owT", [DM, H])
    ob = P("ob", [1, H])
    anw, anb = P("anw", [1, H]), P("anb", [1, H])
    fnw, fnb = P("fnw", [1, H]), P("fnb", [1, H])
    ff1wT = P("ff1wT", [H, I_])
    ff1b = P("ff1b", [32, 128])
    ff2wT = P("ff2wT", [I_, H])
    ffb2 = P("ffb2", [1, H])
    y = nc.declare_dram_parameter("y", [SM, H], F32, isOutput=True)

    with tile.TileContext(nc) as tc:
        cst = tc.alloc_tile_pool(name="cst", bufs=1)
        dram = tc.alloc_tile_pool(name="dram", bufs=1, space="DRAM")
        xmp = tc.alloc_tile_pool(name="xmp", bufs=1)
        setp = tc.alloc_tile_pool(name="setp", bufs=1)
        ps_set = tc.alloc_tile_pool(name="ps_set", bufs=2, space="PSUM")

        ag_in = dram.tile([H, SM], F32)
        ag_out = dram.tile([NC, H, SM], F32, addr_space="Shared")
        rs_in = dram.tile([S, H], F32)
        rs_out = dram.tile([SM, H], F32)

        ident = cst.tile([128, 128], F32)
        make_identity(nc, ident)
        ones_f = cst.tile([1, 128], F32)
        nc.gpsimd.memset(ones_f, 1.0)
        ones1 = cst.tile([1, 128], F32R)
        nc.vector.tensor_copy(ones1[:], ones_f[:])
        ones_col = cst.tile([128, 1], F32)
        nc.gpsimd.memset(ones_col, 1.0)

        def load_vec(p):
            t = setp.tile([1, H], F32, tag=f"v_{p.name}")
            nc.sync.dma_start(out=t[:], in_=p[:])
            return t

        vecs = {n: load_vec(p) for n, p in
                [("anw", anw), ("anb", anb), ("fnw", fnw), ("fnb", fnb),
                 ("ob", ob), ("ffb2", ffb2)]}

        def bcast(name, pool=None):
            v = vecs[name]
            bc = (pool or cst).tile([128, H], F32, tag=f"bc_{name}", name=f"bc_{name}")
            for hf in range(H // 512):
                ps = ps_set.tile([128, 512], F32)
                nc.tensor.matmul(ps[:], ones_f[0:1, :], v[0:1, hf * 512:(hf + 1) * 512],
                                 start=True, stop=True)
                nc.vector.tensor_copy(bc[:, hf * 512:(hf + 1) * 512], ps[:])
            return bc

        anw_bc, anb_bc = bcast("anw", setp), bcast("anb", setp)
        fnw_bc, fnb_bc = bcast("fnw"), bcast("fnb")
        ob_bc, ffb2_bc = bcast("ob"), bcast("ffb2")

        bqkv_sb = setp.tile([1, 3 * DM], F32)
        nc.sync.dma_start(out=bqkv_sb[:], in_=bqkv[:])
        qkvb_pp = []
        for j in range(3):
            ps = ps_set.tile([128, 512], F32)
            nc.tensor.matmul(ps[:, 0:1],
                             bqkv_sb[0:1, j * 128:(j + 1) * 128],
                             ones_f[0:1, 0:1], start=True, stop=True)
            t = cst.tile([128, 1], F32, tag=f"b_pp{j}")
            nc.vector.tensor_copy(t[:], ps[:, 0:1])
            qkvb_pp.append(t)

        ffb1_ld = setp.tile([32, 128], F32)
        nc.sync.dma_start(out=ffb1_ld[:], in_=ff1b[:])
        ps = ps_set.tile([128, 512], F32)
        nc.tensor.transpose(ps[:, 0:32], ffb1_ld[:], ident[0:32, 0:32])
        ffb1_pp = cst.tile([128, 32], F32)
        nc.vector.tensor_copy(ffb1_pp[:], ps[:, 0:32])

        xm_tiles = []
        for i in range(4):
            t = xmp.tile([128, H], F32, tag=f"xm{i}")
            nc.sync.dma_start(out=t[:], in_=x_m[i * 128:(i + 1) * 128, :])
            xm_tiles.append(t)

        with tc.tile_pool(name="ln1", bufs=1) as lnp, \
             tc.tile_pool(name="ln1s", bufs=3) as lnsp:
            xn_tiles = []
            for i in range(4):
                ns = lnsp.tile([128, 1], F32, tag="negsum")
                nc.vector.reduce_sum(out=ns[:], in_=xm_tiles[i][:], axis=AXX,
                                     negate=True)
                nm = lnsp.tile([128, 1], F32, tag="negmean")
                nc.scalar.mul(nm[:], ns[:], 1.0 / H)
                xn = lnp.tile([128, H], F32, tag=f"xn{i}")
                nc.vector.scalar_tensor_tensor(
                    out=xn[:], in0=xm_tiles[i][:], scalar=nm[:], in1=anw_bc[:],
                    op0=ALU.add, op1=ALU.mult)
                nc.vector.tensor_add(xn[:], xn[:], anb_bc[:])
                xn_tiles.append(xn)
            for hc in range(HC):
                xt = lnp.tile([128, SM], F32, tag=f"xnT{hc}")
                for si in range(4):
                    ps = ps_set.tile([128, 512], F32, tag="tps")
                    nc.tensor.transpose(ps[:, 0:128],
                                        xn_tiles[si][:, hc * 128:(hc + 1) * 128],
                                        ident[:])
                    nc.vector.tensor_copy(xt[:, si * 128:(si + 1) * 128],
                                          ps[:, 0:128])
                nc.sync.dma_start(out=ag_in[hc * 128:(hc + 1) * 128, :], in_=xt[:])
            nc.gpsimd.collective_compute(
                "AllGather", ALU.bypass, replica_groups=[list(range(NC))],
                ins=[ag_in.opt()], outs=[ag_out.opt()])
        ps_set.release()
        setp.release()

        with tc.tile_pool(name="attn", bufs=1) as at, \
             tc.tile_pool(name="stream", bufs=10) as stp, \
             tc.tile_pool(name="vtp", bufs=3) as vtp, \
             tc.tile_pool(name="expp", bufs=6) as expp, \
             tc.tile_pool(name="rcp", bufs=2) as rcp, \
             tc.tile_pool(name="aop", bufs=3) as aop, \
             tc.tile_pool(name="ps_mm", bufs=3, space="PSUM") as ps_mm, \
             tc.tile_pool(name="ps_acc", bufs=1, space="PSUM") as ps_acc, \
             tc.tile_pool(name="ps_bc", bufs=1, space="PSUM") as ps_bc:

            wqkv_t = []
            for hc in range(HC):
                t = at.tile([128, 3 * DM], F32R, tag=f"wqkv{hc}")
                nc.sync.dma_start(out=t[:],
                                  in_=wqkvT[hc * 128:(hc + 1) * 128, :].bitcast(F32R))
                wqkv_t.append(t)
            owT_sb = at.tile([DM, H], F32R, tag="owT")
            nc.sync.dma_start(out=owT_sb[:], in_=owT[:].bitcast(F32R))

            QTp = [at.tile([128, S], BF16, tag=f"QTp{h}", name=f"QTp{h}")
                   for h in range(2)]
            KTp = [at.tile([128, S], BF16, tag=f"KTp{h}", name=f"KTp{h}")
                   for h in range(2)]
            for h in range(2):
                z = slice(HD, 128) if h == 0 else slice(0, HD)
                nc.gpsimd.memset(QTp[h][z, :], 0.0)
                nc.gpsimd.memset(KTp[h][z, :], 0.0)
            vaug = [[at.tile([128, HD + 1], BF16, tag=f"va{h}_{t}", name=f"va{h}_{t}")
                     for t in range(32)] for h in range(2)]
            for h in range(2):
                for t in range(32):
                    nc.vector.tensor_copy(vaug[h][t][:, HD:HD + 1], ones_col[:])

            for r in range(ST):
                xnr = []
                for hc in range(HC):
                    t = stp.tile([128, 512], F32R, tag="xnr")
                    nc.sync.dma_start(
                        out=t[:], in_=ag_out[r, hc * 128:(hc + 1) * 128, :].bitcast(F32R))
                    xnr.append(t)
                for j, dest in ((0, QTp), (1, KTp)):
                    ps = ps_mm.tile([128, 512], F32, tag="mm")
                    for hc in range(HC):
                        nc.tensor.matmul(ps[:], wqkv_t[hc][:, j * 128:(j + 1) * 128],
                                         xnr[hc][:], start=(hc == 0), stop=(hc == 7))
                    for h in range(2):
                        hs = slice(h * HD, (h + 1) * HD)
                        nc.vector.tensor_scalar_add(
                            dest[h][hs, r * 512:(r + 1) * 512], ps[hs, :],
                            qkvb_pp[j][hs, :])
                ps = ps_mm.tile([128, 512], F32, tag="mm")
                for hc in range(HC):
                    nc.tensor.matmul(ps[:], wqkv_t[hc][:, 2 * 128:3 * 128],
                                     xnr[hc][:], start=(hc == 0), stop=(hc == 7))
                vtmp = vtp.tile([128, 512], F32, tag="vtmp")
                nc.vector.tensor_scalar_add(vtmp[:], ps[:], qkvb_pp[2][:])
                for tb in range(4):
                    pst = ps_bc.tile([128, 128], F32, tag="vtp")
                    nc.tensor.transpose(pst[:], vtmp[:, tb * 128:(tb + 1) * 128],
                                        ident[:])
                    ti = r * 4 + tb
                    nc.vector.tensor_copy(vaug[0][ti][:, 0:HD], pst[:, 0:HD])
                    nc.vector.tensor_copy(vaug[1][ti][:, 0:HD], pst[:, HD:2 * HD])

            ctxT = at.tile([128, S], F32R, tag="ctxT")
            chains = [(h, r) for h in range(2) for r in range(ST)]
            for g in range(0, len(chains), 3):
                grp = chains[g:g + 3]
                cps_l = [ps_acc.tile([128, 512], F32, name=f"cps{g}_{i}",
                                     tag=f"cps{i}")
                         for i in range(len(grp))]
                for t in range(32):
                    exl = []
                    for (h, r), cps in zip(grp, cps_l):
                        sps = ps_mm.tile([128, 512], F32, tag="mm")
                        nc.tensor.matmul(sps[:],
                                         KTp[h][:, t * 128:(t + 1) * 128],
                                         QTp[h][:, r * 512:(r + 1) * 512],
                                         start=True, stop=True)
                        ex = expp.tile([128, 512], BF16, tag="exp")
                        nc.scalar.activation(ex[:], sps[:], AF.Exp, scale=0.125)
                        exl.append(ex)
                    for (h, r), cps, ex in zip(grp, cps_l, exl):
                        nc.tensor.matmul(cps[0:HD + 1, :], vaug[h][t][:, :],
                                         ex[:], start=(t == 0), stop=(t == 31),
                                         skip_group_check=True)
                for (h, r), cps in zip(grp, cps_l):
                    hs = slice(h * HD, (h + 1) * HD)
                    rc = rcp.tile([1, 512], F32R, tag="rc")
                    with nc.allow_low_precision(reason="f32r softmax denom"):
                        nc.vector.reciprocal(rc[:], cps[HD:HD + 1, :])
                    bps = ps_bc.tile([128, 512], F32, tag="rbc")
                    nc.tensor.matmul(bps[0:HD, :], ones1[0:1, 0:HD], rc[0:1, :],
                                     start=True, stop=True)
                    bsb = rcp.tile([HD, 512], F32, tag="bsb")
                    nc.vector.tensor_copy(bsb[:], bps[0:HD, :])
                    nc.vector.tensor_mul(ctxT[hs, r * 512:(r + 1) * 512],
                                         cps[0:HD, :], bsb[:])

            for s128 in range(32):
                ao = aop.tile([128, H], F32, tag="ao")
                for hf in range(2):
                    ps = ps_mm.tile([128, 512], F32, tag="mm")
                    nc.tensor.matmul(ps[:], ctxT[:, s128 * 128:(s128 + 1) * 128],
                                     owT_sb[:, hf * 512:(hf + 1) * 512],
                                     start=True, stop=True)
                    nc.vector.tensor_copy(ao[:, hf * 512:(hf + 1) * 512], ps[:])
                nc.sync.dma_start(out=rs_in[s128 * 128:(s128 + 1) * 128, :], in_=ao[:])

        nc.gpsimd.collective_compute(
            "ReduceScatter", ALU.add, replica_groups=[list(range(NC))],
            ins=[rs_in.opt()], outs=[rs_out.opt()])

        with tc.tile_pool(name="ff", bufs=1) as ff, \
             tc.tile_pool(name="ffs", bufs=3) as ffsp, \
             tc.tile_pool(name="w1p", bufs=9) as w1p, \
             tc.tile_pool(name="w2p", bufs=4) as w2p, \
             tc.tile_pool(name="ps_f1", bufs=2, space="PSUM") as ps_f1, \
             tc.tile_pool(name="ps_f2", bufs=1, space="PSUM") as ps_f2:

            ln2p = tc.alloc_tile_pool(name="ln2p", bufs=1)
            x2_tiles, xn2_tiles = [], []
            for i in range(4):
                rl = ffsp.tile([128, H], F32, tag="rsld")
                nc.sync.dma_start(out=rl[:], in_=rs_out[i * 128:(i + 1) * 128, :])
                x2 = ff.tile([128, H], F32, tag=f"x2{i}")
                nc.vector.tensor_add(x2[:], rl[:], xm_tiles[i][:])
                nc.vector.tensor_add(x2[:], x2[:], ob_bc[:])
                x2_tiles.append(x2)
                ns = ffsp.tile([128, 1], F32, tag="negsum2")
                nc.vector.reduce_sum(out=ns[:], in_=x2[:], axis=AXX, negate=True)
                nm = ffsp.tile([128, 1], F32, tag="negmean2")
                nc.scalar.mul(nm[:], ns[:], 1.0 / H)
                xn2 = ln2p.tile([128, H], F32, tag=f"xn2{i}")
                nc.vector.scalar_tensor_tensor(
                    out=xn2[:], in0=x2[:], scalar=nm[:], in1=fnw_bc[:],
                    op0=ALU.add, op1=ALU.mult)
                nc.vector.tensor_add(xn2[:], xn2[:], fnb_bc[:])
                xn2_tiles.append(xn2)

            xn2T = []
            for hc in range(HC):
                xt = ff.tile([128, SM], F32R, tag=f"xn2T{hc}")
                for si in range(4):
                    ps = ps_f1.tile([128, 512], F32, tag="f1")
                    nc.tensor.transpose(ps[:, 0:128],
                                        xn2_tiles[si][:, hc * 128:(hc + 1) * 128],
                                        ident[:])
                    nc.vector.tensor_copy(xt[:, si * 128:(si + 1) * 128],
                                          ps[:, 0:128])
                xn2T.append(xt)
            ln2p.release()

            hT = [ff.tile([128, SM], F32R, tag=f"hT{i}", name=f"hT{i}") for i in range(32)]
            for ib in range(8):
                w1t = []
                for hc in range(HC):
                    t = w1p.tile([128, 512], F32R, tag="w1")
                    nc.sync.dma_start(
                        out=t[:],
                        in_=ff1wT[hc * 128:(hc + 1) * 128,
                                  ib * 512:(ib + 1) * 512].bitcast(F32R))
                    w1t.append(t)
                for sub in range(4):
                    it = ib * 4 + sub
                    ps = ps_f1.tile([128, 512], F32, tag="f1")
                    for hc in range(HC):
                        nc.tensor.matmul(ps[:],
                                         w1t[hc][:, sub * 128:(sub + 1) * 128],
                                         xn2T[hc][:], start=(hc == 0), stop=(hc == 7))
                    nc.scalar.activation(hT[it][:], ps[:], AF.Relu,
                                         bias=ffb1_pp[:, it:it + 1])

            y_sb = [ff.tile([128, H], F32, tag=f"y{i}", name=f"ysb{i}") for i in range(4)]
            for hf in range(2):
                yps = [ps_f2.tile([128, 512], F32, name=f"yps{hf}_{i}", tag=f"yps{i}", bufs=1) for i in range(4)]
                for ic in range(32):
                    w2t = w2p.tile([128, 512], F32R, tag="w2")
                    nc.sync.dma_start(
                        out=w2t[:],
                        in_=ff2wT[ic * 128:(ic + 1) * 128,
                                  hf * 512:(hf + 1) * 512].bitcast(F32R))
                    for s4 in range(4):
                        nc.tensor.matmul(yps[s4][:],
                                         hT[ic][:, s4 * 128:(s4 + 1) * 128],
                                         w2t[:], start=(ic == 0), stop=(ic == 31),
                                         skip_group_check=True)
                for s4 in range(4):
                    sl = slice(hf * 512, (hf + 1) * 512)
                    nc.vector.tensor_add(y_sb[s4][:, sl], yps[s4][:],
                                         x2_tiles[s4][:, sl])
                    nc.vector.tensor_add(y_sb[s4][:, sl], y_sb[s4][:, sl],
                                         ffb2_bc[:, sl])
            for s4 in range(4):
                nc.sync.dma_start(out=y[s4 * 128:(s4 + 1) * 128, :], in_=y_sb[s4][:])

        xmp.release()
        dram.release()
        cst.release()

    nc.compile()
    return nc


def make_in_maps(inputs):
    f = lambda a: np.ascontiguousarray(np.asarray(a, dtype=np.float32))
    x = f(inputs["x"])
    q_w, k_w, v_w = f(inputs["q_w"]), f(inputs["k_w"]), f(inputs["v_w"])
    o_w = f(inputs["o_w"])
    ff1_w, ff2_w = f(inputs["ff1_w"]), f(inputs["ff2_w"])
    ff1wT = np.ascontiguousarray(ff1_w.T)
    ff2wT = np.ascontiguousarray(ff2_w.T)
    ff1b = np.ascontiguousarray(f(inputs["ff1_b"]).reshape(32, 128))
    row = lambda a: np.ascontiguousarray(a.reshape(1, -1))
    in_maps = []
    for m in range(NC):
        dm = slice(m * DM, (m + 1) * DM)
        wqkvT = np.ascontiguousarray(
            np.concatenate([q_w[dm].T, k_w[dm].T, v_w[dm].T], axis=1))
        bqkv = np.ascontiguousarray(np.concatenate(
            [f(inputs["q_b"])[dm], f(inputs["k_b"])[dm], f(inputs["v_b"])[dm]]
        ).reshape(1, -1))
        in_maps.append({
            "x_m": np.ascontiguousarray(x[m * SM:(m + 1) * SM]),
            "wqkvT": wqkvT,
            "bqkv": bqkv,
            "owT": np.ascontiguousarray(o_w[:, dm].T),
            "ob": row(f(inputs["o_b"])),
            "anw": row(f(inputs["an_w"])), "anb": row(f(inputs["an_b"])),
            "fnw": row(f(inputs["fn_w"])), "fnb": row(f(inputs["fn_b"])),
            "ff1wT": ff1wT, "ff1b": ff1b,
            "ff2wT": ff2wT, "ffb2": row(f(inputs["ff2_b"])),
        })
    return in_maps


def kernel(**inputs) -> np.ndarray:
    from concourse.bass_utils import run_bass_kernel_spmd
    if "nc" not in _CACHE:
        _CACHE["nc"] = build_nc()
    nc = _CACHE["nc"]
    in_maps = make_in_maps(inputs)
    res = run_bass_kernel_spmd(nc, in_maps, core_ids=list(range(NC)))
    return np.concatenate([res.results[m]["y"] for m in range(NC)], axis=0)



# revision 11
# speedup vs baseline: 1.0937x; 1.0937x over previous
import sys

sys.path.insert(0, '/opt/trn_rl_repo')

import numpy as np
import ml_dtypes
import concourse.bass as bass
from concourse import bacc
import concourse.mybir as mybir
import concourse.tile as tile
from concourse.masks import make_identity

S = 4096
H = 1024
I_ = 4096
NH = 16
HD = 64
NC = 8
SM = S // NC
DM = 128
HC = H // 128
ST = S // 512
NT = S // 128
F32 = mybir.dt.float32
F32R = mybir.dt.float32r
BF16 = mybir.dt.bfloat16
AF = mybir.ActivationFunctionType
ALU = mybir.AluOpType

_CACHE = {}


def build_nc():
    nc = bacc.Bacc(None, target_bir_lowering=False, debug=False)

    def P(name, shape, dt=F32):
        return nc.declare_dram_parameter(name, shape, dt, isOutput=False)

    x_p = P("x", [S, H])
    x_own = P("x_own", [SM, H])
    wqkvT = P("wqkvT", [H, 3 * DM], BF16)
    bqkv = P("bqkv", [1, 3 * DM])
    owT = P("owT", [DM, H], BF16)
    ob = P("ob", [1, H])
    ff1w3 = P("ff1w3", [32, 128, H], BF16)
    ff1b = P("ff1b", [32, 128])
    ff2wT = P("ff2wT", [I_, H], BF16)
    ffb2 = P("ffb2", [1, H])
    y = nc.declare_dram_parameter("y", [SM, H], F32, isOutput=True)

    with tile.TileContext(nc) as tc:
        cst = tc.alloc_tile_pool(name="cst", bufs=1)
        dram = tc.alloc_tile_pool(name="dram", bufs=1, space="DRAM")
        setp = tc.alloc_tile_pool(name="setp", bufs=1)
        ps_set = tc.alloc_tile_pool(name="ps_set", bufs=2, space="PSUM")

        rs_in = dram.tile([S, H], BF16)
        rs_out = dram.tile([SM, H], BF16)

        ones_f = cst.tile([1, 128], F32)
        nc.gpsimd.memset(ones_f, 1.0)
        ones_r = cst.tile([1, 128], F32R)
        nc.vector.tensor_copy(ones_r[:], ones_f[:])

        def load_vec(p):
            t = setp.tile([1, H], F32, tag=f"v_{p.name}")
            nc.sync.dma_start(out=t[:], in_=p[:])
            return t

        ob_v, ffb2_v = load_vec(ob), load_vec(ffb2)

        def bcast(v, name):
            bc = cst.tile([128, H], F32, tag=f"bc_{name}", name=f"bc_{name}")
            for hf in range(H // 512):
                ps = ps_set.tile([128, 512], F32)
                nc.tensor.matmul(ps[:], ones_f[0:1, :], v[0:1, hf * 512:(hf + 1) * 512],
                                 start=True, stop=True)
                nc.vector.tensor_copy(bc[:, hf * 512:(hf + 1) * 512], ps[:])
            return bc

        ob_bc = bcast(ob_v, "ob")
        ffb2_bc = bcast(ffb2_v, "ffb2")

        bqkv_sb = setp.tile([1, 3 * DM], F32)
        nc.sync.dma_start(out=bqkv_sb[:], in_=bqkv[:])
        qkvb_pp = []
        for j in range(3):
            ps = ps_set.tile([128, 512], F32)
            nc.tensor.matmul(ps[:, 0:1],
                             bqkv_sb[0:1, j * 128:(j + 1) * 128],
                             ones_f[0:1, 0:1], start=True, stop=True)
            t = cst.tile([128, 1], F32, tag=f"b_pp{j}")
            nc.vector.tensor_copy(t[:], ps[:, 0:1])
            qkvb_pp.append(t)

        ident_b = cst.tile([128, 128], BF16)
        make_identity(nc, ident_b)
        ident_f = cst.tile([32, 32], F32)
        make_identity(nc, ident_f)
        ffb1_ld = setp.tile([32, 128], F32)
        nc.sync.dma_start(out=ffb1_ld[:], in_=ff1b[:])
        ps = ps_set.tile([128, 512], F32)
        nc.tensor.transpose(ps[:, 0:32], ffb1_ld[:], ident_f[:])
        ffb1_pp = cst.tile([128, 32], F32)
        nc.vector.tensor_copy(ffb1_pp[:], ps[:, 0:32])

        wq_sb = []
        for hc in range(HC):
            t = cst.tile([128, 3 * DM], BF16, tag=f"wqkv{hc}")
            nc.sync.dma_start(out=t[:], in_=wqkvT[hc * 128:(hc + 1) * 128, :])
            wq_sb.append(t)
        owT_sb = cst.tile([DM, H], BF16, tag="owT")
        nc.sync.dma_start(out=owT_sb[:], in_=owT[:])

        ps_set.release()
        setp.release()

        ff = tc.alloc_tile_pool(name="ff", bufs=1)
        attnp = tc.alloc_tile_pool(name="attnp", bufs=1)
        xntp = tc.alloc_tile_pool(name="xntp", bufs=1)

        xnT = xntp.tile([128, HC, S], BF16, name="xnT")
        with tc.tile_pool(name="xp", bufs=4) as xp, \
             tc.tile_pool(name="xcp", bufs=4) as xcp, \
             tc.tile_pool(name="lns", bufs=8) as lns:
            for t in range(NT):
                xt = xp.tile([128, H], F32, tag="x")
                nc.sync.dma_start(out=xt[:], in_=x_p[t * 128:(t + 1) * 128, :])
                xc = xcp.tile([128, H], BF16, tag="xc")
                sums = lns.tile([128, 1], F32, tag="sums")
                nc.scalar.activation(xc[:], xt[:], AF.Copy, accum_out=sums[:])
                nm = lns.tile([128, 1], F32, tag="nm")
                nc.scalar.mul(nm[:], sums[:], -1.0 / H)
                xn = xcp.tile([128, H], BF16, tag="xn")
                nc.vector.tensor_scalar_add(xn[:], xc[:], nm[:])
                nc.sync.dma_start_transpose(
                    out=xnT[:, :, t * 128:(t + 1) * 128], in_=xn[:])

        QTp = attnp.tile([128, S], BF16, name="QTp")
        KTp = attnp.tile([128, S], BF16, name="KTp")
        vb = attnp.tile([128, NT, 130], BF16, name="vb")
        nc.gpsimd.memset(vb[:, :, 64:65], 1.0)
        nc.gpsimd.memset(vb[:, :, 129:130], 1.0)

        with tc.tile_pool(name="ps_qkv", bufs=2, space="PSUM") as ps_qkv, \
             tc.tile_pool(name="vt_sb", bufs=3) as vt_sb:
            for r in range(ST):
                sl = slice(r * 512, (r + 1) * 512)
                qk = ps_qkv.tile([128, 1024], F32, tag="qk")
                for j, dest in ((0, QTp), (1, KTp)):
                    for hc in range(HC):
                        nc.tensor.matmul(qk[:, j * 512:(j + 1) * 512],
                                         wq_sb[hc][:, j * 128:(j + 1) * 128],
                                         xnT[:, hc, sl],
                                         start=(hc == 0), stop=(hc == 7))
                    nc.vector.tensor_scalar_add(
                        dest[:, sl], qk[:, j * 512:(j + 1) * 512], qkvb_pp[j][:])
                vps = ps_qkv.tile([128, 512], F32, tag="v")
                for hc in range(HC):
                    nc.tensor.matmul(vps[:],
                                     wq_sb[hc][:, 2 * 128:3 * 128],
                                     xnT[:, hc, sl],
                                     start=(hc == 0), stop=(hc == 7))
                vt = vt_sb.tile([128, 512], BF16, tag="vt")
                nc.vector.tensor_scalar_add(vt[:], vps[:], qkvb_pp[2][:])
                for tb in range(4):
                    ti = r * 4 + tb
                    vtp = ps_qkv.tile([128, 128], BF16, tag="vtp")
                    nc.tensor.transpose(vtp[:], vt[:, tb * 128:(tb + 1) * 128],
                                        ident_b[:])
                    for h in range(2):
                        nc.vector.tensor_copy(vb[:, ti, h * 65:h * 65 + 64],
                                              vtp[:, h * 64:(h + 1) * 64])

        xntp.release()

        xn2T = ff.tile([128, HC, SM], BF16, name="xn2T")
        x2_t = []

        with tc.tile_pool(name="ps_sps", bufs=2, space="PSUM") as ps_sps, \
             tc.tile_pool(name="ps_ctx", bufs=2, space="PSUM") as ps_ctx, \
             tc.tile_pool(name="exp", bufs=3) as expp, \
             tc.tile_pool(name="ctxp", bufs=3) as ctxp, \
             tc.tile_pool(name="aop", bufs=2) as aop, \
             tc.tile_pool(name="rcp", bufs=8) as rcp, \
             tc.tile_pool(name="ffs", bufs=2) as ffs:
            for r in range(ST):
                sl = slice(r * 512, (r + 1) * 512)
                cps = [ps_ctx.tile([65, 512], F32, tag=f"cps{h}",
                                   name=f"cps{r}_{h}") for h in range(2)]
                for t in range(NT):
                    sps = ps_sps.tile([128, 1024], F32, tag="sps")
                    for h in range(2):
                        hs = slice(h * 64, (h + 1) * 64)
                        nc.tensor.matmul(sps[:, h * 512:(h + 1) * 512],
                                         KTp[hs, t * 128:(t + 1) * 128],
                                         QTp[hs, sl], start=True, stop=True)
                    ex = expp.tile([128, 1024], BF16, tag="ex")
                    nc.scalar.activation(ex[:], sps[:], AF.Exp, scale=0.125)
                    for h in range(2):
                        nc.tensor.matmul(cps[h][:],
                                         vb[:, t, h * 65:(h + 1) * 65],
                                         ex[:, h * 512:(h + 1) * 512],
                                         start=(t == 0), stop=(t == NT - 1),
                                         skip_group_check=True)
                ctxT = ctxp.tile([128, 512], BF16, tag="ctxT", name=f"ctxT{r}")
                for h in range(2):
                    rc = rcp.tile([1, 512], F32R, tag="rc")
                    with nc.allow_low_precision(reason="f32r softmax denom"):
                        nc.vector.reciprocal(rc[:], cps[h][64:65, :])
                    bps = ps_sps.tile([128, 1024], F32, tag="sps",
                                      name=f"bps{r}_{h}")
                    nc.tensor.matmul(bps[0:64, 0:512], ones_r[0:1, 0:64],
                                     rc[0:1, :], start=True, stop=True)
                    bsb = rcp.tile([64, 512], F32, tag="bsb")
                    nc.vector.tensor_copy(bsb[:], bps[0:64, 0:512])
                    nc.vector.tensor_mul(ctxT[h * 64:(h + 1) * 64, :],
                                         cps[h][0:64, :], bsb[:])
                for j in range(4):
                    ops = ps_sps.tile([128, 1024], F32, tag="sps", name=f"ops{r}_{j}")
                    ao = aop.tile([128, H], BF16, tag="ao")
                    for hf in range(2):
                        nc.tensor.matmul(ops[:, hf * 512:(hf + 1) * 512],
                                         ctxT[:, j * 128:(j + 1) * 128],
                                         owT_sb[:, hf * 512:(hf + 1) * 512],
                                         start=True, stop=True)
                        nc.vector.tensor_copy(ao[:, hf * 512:(hf + 1) * 512],
                                              ops[:, hf * 512:(hf + 1) * 512])
                    nc.sync.dma_start(
                        out=rs_in[r * 512 + j * 128:r * 512 + (j + 1) * 128, :],
                        in_=ao[:])
                nc.gpsimd.collective_compute(
                    "ReduceScatter", ALU.add, replica_groups=[list(range(NC))],
                    ins=[rs_in[r * 512:(r + 1) * 512, :]],
                    outs=[rs_out[r * 64:(r + 1) * 64, :]])

                rl = ffs.tile([64, H], BF16, tag="rl")
                nc.sync.dma_start(out=rl[:], in_=rs_out[r * 64:(r + 1) * 64, :])
                rf = ffs.tile([64, H], F32, tag="rf")
                nc.vector.tensor_copy(rf[:], rl[:])
                xo = ffs.tile([64, H], F32, tag="xo")
                nc.sync.dma_start(out=xo[:], in_=x_own[r * 64:(r + 1) * 64, :])
                x2 = ff.tile([64, H], F32, tag=f"x2_{r}", name=f"x2_{r}")
                nc.vector.tensor_add(x2[:], rf[:], xo[:])
                nc.vector.tensor_add(x2[:], x2[:], ob_bc[0:64, :])
                x2_t.append(x2)
                x2c = ffs.tile([64, H], BF16, tag="x2c")
                sums = ffs.tile([64, 1], F32, tag="s2")
                nc.scalar.activation(x2c[:], x2[:], AF.Copy, accum_out=sums[:])
                nm = ffs.tile([64, 1], F32, tag="nm2")
                nc.scalar.mul(nm[:], sums[:], -1.0 / H)
                xn2 = ffs.tile([64, H], BF16, tag="xn2")
                nc.vector.tensor_scalar_add(xn2[:], x2c[:], nm[:])
                nc.sync.dma_start_transpose(
                    out=xn2T[:, :, r * 64:(r + 1) * 64], in_=xn2[:])

        attnp.release()

        ffl = tc.alloc_tile_pool(name="ffl", bufs=1)
        with tc.tile_pool(name="w1p", bufs=8) as w1p, \
             tc.tile_pool(name="w2p", bufs=8) as w2p, \
             tc.tile_pool(name="yp", bufs=3) as yp, \
             tc.tile_pool(name="ps_f1", bufs=2, space="PSUM") as ps_f1, \
             tc.tile_pool(name="ps_f2", bufs=1, space="PSUM") as ps_f2:

            hT = [ffl.tile([128, SM], BF16, tag=f"hT{i}", name=f"hT{i}")
                  for i in range(32)]
            for it in range(32):
                w1t = w1p.tile([128, HC, 128], BF16, tag="w1")
                nc.sync.dma_start(out=w1t[:], in_=ff1w3[it, :, :])
                ps1 = ps_f1.tile([128, 512], F32, tag="f1")
                for hc in range(HC):
                    nc.tensor.matmul(ps1[:], w1t[:, hc, :], xn2T[:, hc, :],
                                     start=(hc == 0), stop=(hc == 7))
                nc.scalar.activation(hT[it][:], ps1[:], AF.Relu,
                                     bias=ffb1_pp[:, it:it + 1])

            for hf in range(2):
                sl = slice(hf * 512, (hf + 1) * 512)
                yps = [ps_f2.tile([128, 512], F32, name=f"yps{hf}_{i}",
                                  tag=f"yps{i}", bufs=1) for i in range(4)]
                for ic in range(32):
                    w2t = w2p.tile([128, 512], BF16, tag="w2")
                    nc.sync.dma_start(
                        out=w2t[:],
                        in_=ff2wT[ic * 128:(ic + 1) * 128,
                                  hf * 512:(hf + 1) * 512])
                    for s4 in range(4):
                        nc.tensor.matmul(yps[s4][:],
                                         hT[ic][:, s4 * 128:(s4 + 1) * 128],
                                         w2t[:], start=(ic == 0), stop=(ic == 31),
                                         skip_group_check=True)
                for s4 in range(4):
                    for half in range(2):
                        r = s4 * 2 + half
                        hsl = slice(half * 64, (half + 1) * 64)
                        yt = yp.tile([64, 512], F32, tag="yt")
                        nc.vector.tensor_add(yt[:], yps[s4][hsl, :],
                                             ffb2_bc[0:64, sl])
                        nc.vector.tensor_add(yt[:], yt[:], x2_t[r][:, sl])
                        nc.sync.dma_start(
                            out=y[s4 * 128 + half * 64:s4 * 128 + (half + 1) * 64,
                                  sl],
                            in_=yt[:])

        ffl.release()
        ff.release()
        dram.release()
        cst.release()

    nc.compile()
    return nc


def make_in_maps(inputs):
    f32 = lambda a: np.ascontiguousarray(np.asarray(a, dtype=np.float32))
    bf = lambda a: np.ascontiguousarray(np.asarray(a, dtype=np.float32)
                                        .astype(ml_dtypes.bfloat16))
    x = f32(inputs["x"])
    anw, anb = f32(inputs["an_w"]), f32(inputs["an_b"])
    fnw, fnb = f32(inputs["fn_w"]), f32(inputs["fn_b"])
    q_w, k_w, v_w = f32(inputs["q_w"]), f32(inputs["k_w"]), f32(inputs["v_w"])
    o_w = f32(inputs["o_w"])
    ff1_w, ff2_w = f32(inputs["ff1_w"]), f32(inputs["ff2_w"])

    w1_eff = ff1_w * fnw[None, :]
    b1_eff = f32(inputs["ff1_b"]) + ff1_w @ fnb
    ff1w3 = np.ascontiguousarray(
        w1_eff.reshape(32, 128, HC, 128).transpose(0, 3, 2, 1)
        .reshape(32, 128, H).astype(ml_dtypes.bfloat16))
    ff2wT = bf(ff2_w.T)
    ff1b = np.ascontiguousarray(b1_eff.reshape(32, 128))
    row = lambda a: np.ascontiguousarray(a.reshape(1, -1))

    in_maps = []
    for m in range(NC):
        dm = slice(m * DM, (m + 1) * DM)
        wq = (q_w[dm] * anw[None, :]).T
        wk = (k_w[dm] * anw[None, :]).T
        wv = (v_w[dm] * anw[None, :]).T
        wqkvT = np.ascontiguousarray(
            np.concatenate([wq, wk, wv], axis=1).astype(ml_dtypes.bfloat16))
        bq = f32(inputs["q_b"])[dm] + q_w[dm] @ anb
        bk = f32(inputs["k_b"])[dm] + k_w[dm] @ anb
        bv = f32(inputs["v_b"])[dm] + v_w[dm] @ anb
        in_maps.append({
            "x": x,
            "x_own": np.ascontiguousarray(x[row_perm(m)]),
            "wqkvT": wqkvT,
            "bqkv": row(np.concatenate([bq, bk, bv])),
            "owT": bf(o_w[:, dm].T),
            "ob": row(f32(inputs["o_b"])),
            "ff1w3": ff1w3,
            "ff1b": ff1b,
            "ff2wT": ff2wT,
            "ffb2": row(f32(inputs["ff2_b"])),
        })
    return in_maps


def row_perm(m):
    return np.concatenate(
        [np.arange(r * 512 + m * 64, r * 512 + (m + 1) * 64) for r in range(ST)])


def kernel(**inputs) -> np.ndarray:
    from concourse.bass_utils import run_bass_kernel_spmd
    if "nc" not in _CACHE:
        _CACHE["nc"] = build_nc()
    nc = _CACHE["nc"]
    in_maps = make_in_maps(inputs)
    res = run_bass_kernel_spmd(nc, in_maps, core_ids=list(range(NC)))
    out = np.empty((S, H), dtype=np.float32)
    for m in range(NC):
        out[row_perm(m)] = res.results[m]["y"]
    return out
